# revision 16
# baseline (speedup 1.0000x reference)
"""Trainium2 Bass kernel for nn_BipartiteGCN (gnn_message_passing).

Strategy (derived from the reference dataflow):
  * The final output only consumes concat(skip)[train_ids] (B=4096 rows) and
    the layer-1 edge update is never used, so the computation collapses to
    per-(train-row, slot) work on the 16 incident edges of each sampled node:
    the edge-l0 update for exactly those B*16 slots per branch, then the two
    node updates restricted to the sampled rows.
  * Shard the 4096 train rows across 8 NeuronCores (512 rows/core); zero
    inter-core communication. Feature gathers run on-device (dma_gather /
    indirect DMA) from full replicated feats / edge_emb tables; the host only
    composes integer index arrays and combines weight matrices.
  * On-chip layout is feature-major: gathered rows are PE-transposed once,
    projections are fp32r tensor-engine matmuls, attention logit reductions
    are head-mask matmuls, softmax over s=16 runs packed on [64,512] tiles,
    and aggregation uses strided vector reduces.
"""

import numpy as np

import concourse.bass as bass
import concourse.bacc as bacc
import concourse.mybir as mybir
import concourse.tile as tile
from concourse.masks import make_identity

F32 = mybir.dt.float32
F32R = mybir.dt.float32r
BF16 = mybir.dt.bfloat16
I16 = mybir.dt.int16
I32 = mybir.dt.int32
AL = mybir.AluOpType
ACTF = mybir.ActivationFunctionType

# problem dims
N_NODES, S, E_EDGES, NMP, H, O = 20000, 16, 160000, 2, 4, 64
D_FEAT, E_DIM, PREP, NCLS, B = 64, 32, 128, 8, 4096
D1 = H * O  # 256

N_CORES = 8
RPC = B // N_CORES          # 512 train rows per core
NSEG = 8                    # gather segments per branch


def _wflat(w):
    return np.ascontiguousarray(np.transpose(w, (1, 0, 2)).reshape(w.shape[1], H * O))


def _wrap16_rep(idx):
    """dma_gather index layout: [128, n/16] int16, 16-row wrapped, replicated x8."""
    n = idx.shape[0]
    w = idx.reshape(n // 16, 16).T.astype(np.int16)
    return np.tile(w, (8, 1)).copy()


def build_host_consts(rpc):
    import ml_dtypes
    nch = rpc // 32
    # edge (per-chunk) base masks
    hmb = np.zeros((128, 2 * 4), np.float32)
    exb = np.zeros((4, 2 * 128), np.float32)
    for blk in range(2):
        for dd in range(128):
            h = (blk * 128 + dd) // 64
            hmb[dd, blk * 4 + h] = 1.0
            exb[h, blk * 128 + dd] = 1.0
    # node packed masks
    hm = np.zeros((128, 2, nch, 4 * nch), np.float32)
    ex = np.zeros((4 * nch, 2, nch, 128), np.float32)
    for blk in range(2):
        for dd in range(128):
            h = (blk * 128 + dd) // 64
            for c in range(nch):
                hm[dd, blk, c, 4 * c + h] = 1.0
                ex[4 * c + h, blk, c, dd] = 1.0
    bf = ml_dtypes.bfloat16
    return (hmb.astype(bf), exb.astype(bf),
            hm.reshape(128, -1).astype(bf), ex.reshape(4 * nch, -1).astype(bf))


def build_program(rpc=RPC):
    nch = rpc // 32            # 512-slot chunks per branch
    slots = rpc * S
    nblk = slots // 128
    rb = rpc // 128

    nc = bacc.Bacc("TRN2", target_bir_lowering=False, debug=False,
                   enable_asserts=False, num_devices=N_CORES)

    d = {}
    def din(name, shape, dt):
        d[name] = nc.dram_tensor(name, shape, dt, kind="ExternalInput")
        return d[name]

    din("feats", (N_NODES, D_FEAT), F32)
    din("edge_emb", (NMP, E_EDGES, E_DIM), F32)
    din("w32", (128, 2 * NMP * 2 * 256), F32R)
    din("w64", (64, NMP * 2 * 256), F32R)
    din("w256", (128, 2 * NMP * 2 * 256), BF16)
    din("wg", (128, 4 * 128), BF16)
    din("vg", (128, 1), F32R)
    din("ones1", (1, 128), F32R)
    din("wfc", (128, 4 * NCLS), BF16)
    din("bfc", (NCLS, 1), F32)
    din("hmb", (128, 2 * 4), BF16)
    din("exb", (4, 2 * 128), BF16)
    din("hm", (128, 2 * nch * 4 * nch), BF16)
    din("ex", (4 * nch, 2 * nch * 128), BF16)
    din("embidx", (128, NMP * (slots // 16)), I16)
    din("sub", (128, NMP * nblk), F32)
    din("iota128", (128, 256), F32)
    din("n0idx", (128, NMP * (slots // 16)), I16)
    din("n1idx", (128, NMP * (slots // 16)), I16)
    din("tididx", (128, rpc // 16), I16)

    logits_o = nc.dram_tensor("logits_o", (rpc, NCLS), F32, kind="ExternalOutput")
    gate_o = nc.dram_tensor("gate_o", (NMP, rpc), F32, kind="ExternalOutput")

    with tile.TileContext(nc) as tc:
        _emit(nc, tc, d, logits_o, gate_o, rpc, nch, slots, nblk, rb)

    nc.compile()
    return nc


def _emit(nc, tc, d, logits_o, gate_o, rpc, nch, slots, nblk, rb):
    from contextlib import ExitStack
    ctx = ExitStack()
    cst = ctx.enter_context(tc.tile_pool(name="cst", bufs=1))
    big = ctx.enter_context(tc.tile_pool(name="big", bufs=1))
    gat = ctx.enter_context(tc.tile_pool(name="gat", bufs=1))
    tloc = ctx.enter_context(tc.tile_pool(name="tloc", bufs=2))
    wk = ctx.enter_context(tc.tile_pool(name="wk", bufs=1))
    ps = ctx.enter_context(tc.tile_pool(name="ps", bufs=2, space="PSUM"))
    drp = ctx.enter_context(tc.tile_pool(name="drp", bufs=1, space="DRAM"))

    ident = cst.tile([128, 128], F32)
    make_identity(nc, ident[:])


    def ctile(name, shape, dt):
        t = cst.tile(shape, dt, tag=name)
        nc.sync.dma_start(t[:], d[name][:])
        return t

    w32 = ctile("w32", [128, 2 * NMP * 2 * 256], F32R)
    w64 = ctile("w64", [64, NMP * 2 * 256], F32R)
    w256 = ctile("w256", [128, 2 * NMP * 2 * 256], BF16)
    wg = ctile("wg", [128, 4 * 128], BF16)
    vg = ctile("vg", [128, 1], F32R)
    ones1 = ctile("ones1", [1, 128], F32R)
    wfc = ctile("wfc", [128, 4 * NCLS], BF16)
    bfc = ctile("bfc", [NCLS, 1], F32)
    hmb = ctile("hmb", [128, 2 * 4], BF16)
    exb = ctile("exb", [4, 2 * 128], BF16)
    hm = ctile("hm", [128, 2 * nch * 4 * nch], BF16)
    ex = ctile("ex", [4 * nch, 2 * nch * 128], BF16)
    embidx = ctile("embidx", [128, NMP * (slots // 16)], I16)
    subt = ctile("sub", [128, NMP * nblk], F32)
    iota128 = ctile("iota128", [128, 256], F32)
    n0i = ctile("n0idx", [128, NMP * (slots // 16)], I16)
    n1i = ctile("n1idx", [128, NMP * (slots // 16)], I16)
    tidi = ctile("tididx", [128, rpc // 16], I16)

    def r32(x):
        return x.bitcast(F32R)

    def w32s(kb, b, i, j):
        base = ((kb * NMP + b) * 2 + i) * 256
        return w32[:, base + j * 128: base + (j + 1) * 128]

    def w64s(b, i, j):
        base = (b * 2 + i) * 256
        return w64[:, base + j * 128: base + (j + 1) * 128]

    def w256s(kb, b, i, j):
        base = ((kb * NMP + b) * 2 + i) * 256
        return w256[:, base + j * 128: base + (j + 1) * 128]

    def hms(blk, c):
        return hm[:, (blk * nch + c) * 4 * nch:(blk * nch + c + 1) * 4 * nch]

    def exs(blk, c):
        return ex[:, (blk * nch + c) * 128:(blk * nch + c + 1) * 128]

    NP = 4 * nch
    CPS = nch // NSEG          # chunks per segment

    # ---- FT gather + transpose ----
    ft = wk.tile([128, rb, 64], F32, tag="ft")
    nc.gpsimd.dma_gather(out_ap=ft[:], in_ap=d["feats"][:], idxs_ap=tidi[:],
                         num_idxs=rpc, num_idxs_reg=rpc, elem_size=64)
    ftt = cst.tile([64, rpc], F32R)
    pT = ps.tile([64, rpc], F32, tag="pa", bufs=4)
    for k in range(rb):
        nc.tensor.matmul(pT[:, k * 128:(k + 1) * 128], ft[:, k, :], ident[:],
                         is_transpose=True, start=(k == 0), stop=(k == rb - 1))
    nc.scalar.copy(ftt[:], pT[:])

    f12 = [[None, None], [None, None]]

    for b in range(NMP):
        k0T = big.tile([128, 2, slots], BF16, tag="k0T")
        k1d = drp.tile([128, 2, slots], BF16, tag=f"k1d{b}")

        for seg in range(NSEG):
            sblk = nblk // NSEG          # blocks per segment
            so = seg * sblk
            # ---- segment gathers ----
            oct = gat.tile([128, sblk, 256], F32, tag="oct", bufs=2)
            ioff = b * (slots // 16) + so * 8
            embv = d["edge_emb"][b].rearrange("(u k) d -> u (k d)", k=8)
            nc.gpsimd.dma_gather(out_ap=oct[:], in_ap=embv,
                                 idxs_ap=embidx[:, ioff: ioff + sblk * 8],
                                 num_idxs=sblk * 128, num_idxs_reg=sblk * 128,
                                 elem_size=256)
            f0 = gat.tile([128, sblk, 64], F32, tag="f0", bufs=2)
            nc.gpsimd.dma_gather(out_ap=f0[:], in_ap=d["feats"][:],
                                 idxs_ap=n0i[:, ioff: ioff + sblk * 8],
                                 num_idxs=sblk * 128, num_idxs_reg=sblk * 128,
                                 elem_size=64)
            f1 = gat.tile([128, sblk, 64], F32, tag="f1", bufs=2)
            nc.gpsimd.dma_gather(out_ap=f1[:], in_ap=d["feats"][:],
                                 idxs_ap=n1i[:, ioff: ioff + sblk * 8],
                                 num_idxs=sblk * 128, num_idxs_reg=sblk * 128,
                                 elem_size=64)
            df = gat.tile([128, sblk, 64], F32, tag="df", bufs=2)
            nc.vector.tensor_tensor(df[:], f0[:], f1[:], op=AL.subtract)

            for ci in range(CPS):
                gc = seg * CPS + ci
                cs = slice(gc * 512, (gc + 1) * 512)
                # ---- sub-row select: memb = (iota==sub) * octet ----
                mmk = wk.tile([128, 4, 256], F32, tag="mmk", bufs=2)
                for k in range(4):
                    blki = ci * 4 + k
                    nc.vector.scalar_tensor_tensor(
                        mmk[:, k, :], iota128[:],
                        subt[:, b * nblk + so + blki: b * nblk + so + blki + 1],
                        oct[:, blki, :], op0=AL.is_equal, op1=AL.mult)
                # ---- transposes ----
                pmT = ps.tile([128, 2, 512], F32, tag="pb", bufs=2)
                p64a = ps.tile([64, 512], F32, tag="pa", bufs=4)
                p64b = ps.tile([64, 512], F32, tag="pa", bufs=4)
                for k in range(4):
                    blki = ci * 4 + k
                    for h in range(2):
                        nc.tensor.matmul(pmT[:, h, k * 128:(k + 1) * 128],
                                         mmk[:, k, h * 128:(h + 1) * 128],
                                         ident[:], is_transpose=True,
                                         start=(k == 0), stop=(k == 3))
                    nc.tensor.matmul(p64a[:, k * 128:(k + 1) * 128], f1[:, blki, :],
                                     ident[:], is_transpose=True,
                                     start=(k == 0), stop=(k == 3))
                    nc.tensor.matmul(p64b[:, k * 128:(k + 1) * 128], df[:, blki, :],
                                     ident[:], is_transpose=True,
                                     start=(k == 0), stop=(k == 3))
                mT = tloc.tile([128, 2, 512], F32R, tag="mT")
                nc.scalar.copy(mT[:], pmT[:])
                f1t = tloc.tile([64, 512], F32R, tag="f1t")
                nc.scalar.copy(f1t[:], p64a[:])
                dft = tloc.tile([64, 512], F32R, tag="dft")
                nc.scalar.copy(dft[:], p64b[:])

                # ---- projections ----
                qep = ps.tile([128, 2, 512], F32, tag="pb", bufs=2)
                dtp = ps.tile([128, 2, 512], F32, tag="pb", bufs=2)
                for j in range(2):
                    for kb in range(2):
                        nc.tensor.matmul(qep[:, j, :], w32s(kb, b, 0, j),
                                         mT[:, kb, :], start=(kb == 0), stop=(kb == 1))
                    nc.tensor.matmul(dtp[:, j, :], w64s(b, 0, j), dft[:],
                                     start=True, stop=True)
                dts = wk.tile([128, 2, 512], F32, tag="wf", bufs=3)
                nc.scalar.copy(dts[:], dtp[:])
                pp = wk.tile([128, 2, 512], BF16, tag="wh", bufs=3)
                nc.vector.tensor_tensor(pp[:], qep[:], dts[:], op=AL.mult)
                ldc = ps.tile([4, 512], F32, tag="pa", bufs=4)
                for blk in range(2):
                    nc.tensor.matmul(ldc[:], hmb[:, blk * 4:(blk + 1) * 4],
                                     pp[:, blk, :], start=(blk == 0), stop=(blk == 1))
                a0 = wk.tile([4, 512], BF16, tag="a0", bufs=1)
                nc.scalar.activation(a0[:], ldc[:], ACTF.Sigmoid, scale=0.125)

                ktp = ps.tile([128, 2, 512], F32, tag="pb", bufs=2)
                pre0 = ps.tile([128, 2, 512], F32, tag="pb", bufs=2)
                for j in range(2):
                    for kb in range(2):
                        nc.tensor.matmul(ktp[:, j, :], w32s(kb, b, 1, j),
                                         mT[:, kb, :], start=(kb == 0), stop=(kb == 1))
                        nc.tensor.matmul(pre0[:, j, :], w32s(kb, b, 0, j),
                                         mT[:, kb, :], start=(kb == 0), stop=False)
                    nc.tensor.matmul(pre0[:, j, :], w64s(b, 0, j), f1t[:],
                                     start=False, stop=True)
                nc.scalar.copy(k0T[:, :, cs], ktp[:])
                aep = ps.tile([128, 2, 512], F32, tag="pb", bufs=2)
                for blk in range(2):
                    nc.tensor.matmul(aep[:, blk, :], exb[:, blk * 128:(blk + 1) * 128],
                                     a0[:], start=True, stop=True)
                tt = wk.tile([128, 2, 512], F32, tag="wf", bufs=3)
                nc.vector.tensor_tensor(tt[:], aep[:], dts[:], op=AL.mult)
                pre = wk.tile([128, 2, 512], F32, tag="wf", bufs=3)
                nc.vector.tensor_tensor(pre[:], tt[:], pre0[:], op=AL.add)
                npre = wk.tile([128, 2, 512], F32, tag="wf", bufs=3)
                nc.vector.tensor_scalar_min(npre[:], pre[:], 0.0)
                epre = wk.tile([128, 2, 512], F32, tag="wf", bufs=3)
                nc.scalar.activation(epre[:], npre[:], ACTF.Exp)
                em1 = wk.tile([128, 2, 512], F32, tag="wf", bufs=3)
                nc.vector.tensor_scalar_add(em1[:], epre[:], -1.0)
                e1 = wk.tile([128, 2, 512], BF16, tag="wh", bufs=3)
                nc.vector.scalar_tensor_tensor(e1[:], pre[:], 0.0, em1[:],
                                               op0=AL.max, op1=AL.add)
                k1p = ps.tile([128, 2, 512], F32, tag="pb", bufs=2)
                for j in range(2):
                    for kb in range(2):
                        nc.tensor.matmul(k1p[:, j, :], w256s(kb, b, 0, j),
                                         e1[:, kb, :],
                                         start=(kb == 0), stop=(kb == 1))
                k1c = wk.tile([128, 2, 512], BF16, tag="wh", bufs=3)
                nc.scalar.copy(k1c[:], k1p[:])
                nc.sync.dma_start(k1d[:, :, cs], k1c[:])

        # ================= node layers =================
        GRP = 1024                       # slots per batched vector group
        NGR = slots // GRP
        for l in range(2):
            qp = ps.tile([128, 2, rpc], F32, tag="pb", bufs=2)
            if l == 0:
                for j in range(2):
                    nc.tensor.matmul(qp[:, j, :], w64s(b, 1, j), ftt[:],
                                     start=True, stop=True)
            else:
                for j in range(2):
                    for kb in range(2):
                        nc.tensor.matmul(qp[:, j, :], w256s(kb, b, 1, j),
                                         f12[b][0][:, kb, :],
                                         start=(kb == 0), stop=(kb == 1))
            qf = wk.tile([128, 2, rpc], F32, tag="qf", bufs=1)
            nc.scalar.copy(qf[:], qp[:])
            qh = wk.tile([128, 2, rpc], BF16, tag="qh", bufs=1)
            nc.vector.tensor_copy(qh[:], qf[:])

            def kt_group(g):
                gs = slice(g * GRP, (g + 1) * GRP)
                if l == 0:
                    return k0T[:, :, gs]
                kt = wk.tile([128, 2, GRP], BF16, tag="ktc", bufs=1)
                nc.sync.dma_start(kt[:], k1d[:, :, gs])
                return kt[:]

            ln = ps.tile([NP, 512], F32, tag="pa", bufs=4)
            for g in range(NGR):
                ktv = kt_group(g)
                rw = GRP // 16
                qb = qh[:, :, g * rw:(g + 1) * rw].unsqueeze(3).to_broadcast(
                    [128, 2, rw, 16])
                pr = wk.tile([128, 2, GRP], BF16, tag="pp2", bufs=2)
                nc.vector.tensor_tensor(
                    pr[:].rearrange("p b (r s) -> p b r s", s=16), qb,
                    ktv.rearrange("p b (r s) -> p b r s", s=16), op=AL.mult)
                for cc in range(GRP // 512):
                    c = g * (GRP // 512) + cc
                    for blk in range(2):
                        nc.tensor.matmul(ln[:], hms(blk, c),
                                         pr[:, blk, cc * 512:(cc + 1) * 512],
                                         start=(c == 0 and blk == 0),
                                         stop=(c == nch - 1 and blk == 1))

            lnv = ln[:].rearrange("p (r s) -> p r s", s=16)
            mx = wk.tile([NP, 32], F32, tag="mx", bufs=2)
            nc.vector.tensor_reduce(mx[:], lnv, axis=mybir.AxisListType.X, op=AL.max)
            sub = wk.tile([NP, 512], F32, tag="wf", bufs=3)
            nc.vector.tensor_tensor(sub[:].rearrange("p (r s) -> p r s", s=16), lnv,
                                    mx[:].unsqueeze(2).to_broadcast([NP, 32, 16]),
                                    op=AL.subtract)
            esub = wk.tile([NP, 512], F32, tag="wf", bufs=3)
            nc.scalar.activation(esub[:], sub[:], ACTF.Exp, scale=0.125)
            zs = wk.tile([NP, 32], F32, tag="zs", bufs=2)
            nc.vector.tensor_reduce(zs[:], esub[:].rearrange("p (r s) -> p r s", s=16),
                                    axis=mybir.AxisListType.X, op=AL.add)
            zr = wk.tile([NP, 32], F32, tag="zr", bufs=2)
            nc.vector.reciprocal(zr[:], zs[:])
            pn = wk.tile([NP, 512], BF16, tag="pn", bufs=2)
            nc.vector.tensor_tensor(pn[:].rearrange("p (r s) -> p r s", s=16),
                                    esub[:].rearrange("p (r s) -> p r s", s=16),
                                    zr[:].unsqueeze(2).to_broadcast([NP, 32, 16]),
                                    op=AL.mult)

            agg = wk.tile([128, 2, rpc], F32, tag="agg", bufs=1)
            for g in range(NGR):
                aeh = wk.tile([128, 2, GRP], BF16, tag="aeh2", bufs=2)
                for cc in range(GRP // 512):
                    c = g * (GRP // 512) + cc
                    aep = ps.tile([128, 2, 512], F32, tag="pb", bufs=2)
                    for blk in range(2):
                        nc.tensor.matmul(aep[:, blk, :], exs(blk, c), pn[:],
                                         start=True, stop=True)
                    nc.scalar.copy(aeh[:, :, cc * 512:(cc + 1) * 512], aep[:])
                ak = wk.tile([128, 2, GRP], BF16, tag="pp2", bufs=2)
                nc.vector.tensor_tensor(ak[:], aeh[:], kt_group(g), op=AL.mult)
                rw = GRP // 16
                nc.vector.tensor_reduce(agg[:, :, g * rw:(g + 1) * rw],
                                        ak[:].rearrange("p b (r s) -> p b r s", s=16),
                                        axis=mybir.AxisListType.X, op=AL.add)

            prn = wk.tile([128, 2, rpc], F32, tag="wf", bufs=3)
            nc.vector.tensor_tensor(prn[:], qf[:], agg[:], op=AL.add)
            npre = wk.tile([128, 2, rpc], F32, tag="wf", bufs=3)
            nc.vector.tensor_scalar_min(npre[:], prn[:], 0.0)
            epre = wk.tile([128, 2, rpc], F32, tag="wf", bufs=3)
            nc.scalar.activation(epre[:], npre[:], ACTF.Exp)
            em1 = wk.tile([128, 2, rpc], F32, tag="wf", bufs=3)
            nc.vector.tensor_scalar_add(em1[:], epre[:], -1.0)
            fl = big.tile([128, 2, rpc], BF16, tag=f"f12_{b}_{l}")
            nc.vector.scalar_tensor_tensor(fl[:], prn[:], 0.0, em1[:],
                                           op0=AL.max, op1=AL.add)
            f12[b][l] = fl

    # ================= finale =================
    gsb = []
    for b in range(NMP):
        tp = ps.tile([128, rpc], F32, tag="pb", bufs=2)
        for kb in range(4):
            l, jb = kb // 2, kb % 2
            nc.tensor.matmul(tp[:], wg[:, kb * 128:(kb + 1) * 128],
                             f12[b][l][:, jb, :], start=(kb == 0), stop=(kb == 3))
        th = wk.tile([128, rpc], F32R, tag="wf", bufs=3)
        nc.scalar.activation(th[:], tp[:], ACTF.Tanh)
        gp = ps.tile([1, rpc], F32, tag="pa", bufs=4)
        nc.tensor.matmul(gp[:], vg[:], th[:], start=True, stop=True)
        g = wk.tile([1, rpc], F32, tag=f"g{b}")
        nc.scalar.copy(g[:], gp[:])
        gsb.append(g)

    gd = wk.tile([1, rpc], F32, tag="gd")
    nc.vector.tensor_tensor(gd[:], gsb[0][:], gsb[1][:], op=AL.subtract)
    gate0 = wk.tile([1, rpc], F32, tag="gate0")
    nc.scalar.activation(gate0[:], gd[:], ACTF.Sigmoid)
    gate1 = wk.tile([1, rpc], F32, tag="gate1")
    nc.vector.tensor_scalar(gate1[:], gate0[:], -1.0, 1.0, op0=AL.mult, op1=AL.add)
    nc.sync.dma_start(gate_o[0:1, :], gate0[:])
    nc.sync.dma_start(gate_o[1:2, :], gate1[:])

    ge = []
    g0r = wk.tile([1, rpc], F32R, tag="g0r")
    nc.scalar.copy(g0r[:], gate0[:])
    g1r = wk.tile([1, rpc], F32R, tag="g1r")
    nc.scalar.copy(g1r[:], gate1[:])
    for b in range(NMP):
        gep = ps.tile([128, rpc], F32, tag="pb", bufs=2)
        nc.tensor.matmul(gep[:], ones1[:],
                         g0r[:] if b == 0 else g1r[:], start=True, stop=True)
        gs = wk.tile([128, rpc], BF16, tag=f"ge{b}")
        nc.scalar.copy(gs[:], gep[:])
        ge.append(gs)
    pooled = [None, None]
    for l in range(2):
        t0 = wk.tile([128, 2, rpc], BF16, tag="wh", bufs=3)
        nc.vector.tensor_tensor(t0[:], ge[0][:].unsqueeze(1).to_broadcast([128, 2, rpc]),
                                f12[0][l][:], op=AL.mult)
        t1 = wk.tile([128, 2, rpc], BF16, tag="wh", bufs=3)
        nc.vector.tensor_tensor(t1[:], ge[1][:].unsqueeze(1).to_broadcast([128, 2, rpc]),
                                f12[1][l][:], op=AL.mult)
        pl = wk.tile([128, 2, rpc], BF16, tag=f"pool{l}")
        nc.vector.tensor_tensor(pl[:], t0[:], t1[:], op=AL.add)
        pooled[l] = pl

    lg = ps.tile([NCLS, rpc], F32, tag="pa", bufs=4)
    for kb in range(4):
        l, jb = kb // 2, kb % 2
        nc.tensor.matmul(lg[:], wfc[:, kb * NCLS:(kb + 1) * NCLS],
                         pooled[l][:, jb, :], start=(kb == 0), stop=(kb == 3))
    lgb = wk.tile([NCLS, rpc], F32, tag="lgb")
    nc.vector.tensor_scalar_add(lgb[:], lg[:], bfc[:, 0:1])

    rbk = rpc // 128
    lgT = wk.tile([128, rbk * NCLS], F32, tag="lgT")
    plg = ps.tile([128, rbk * NCLS], F32, tag="pa", bufs=4)
    for k in range(rbk):
        nc.tensor.matmul(plg[:, k * NCLS:(k + 1) * NCLS], lgb[:, k * 128:(k + 1) * 128],
                         ident[0:NCLS, 0:NCLS], is_transpose=True,
                         start=(k == 0), stop=(k == rbk - 1))
    nc.scalar.copy(lgT[:], plg[:])
    nc.sync.dma_start(logits_o[:].rearrange("(k p) c -> p k c", p=128),
                      lgT[:].rearrange("p (k c) -> p k c", c=NCLS))

    ctx.close()


def host_prep(inputs, rpc=RPC, n_cores=N_CORES):
    feats = np.ascontiguousarray(np.asarray(inputs["feats"], np.float32))
    emb = np.ascontiguousarray(np.asarray(inputs["edge_emb"], np.float32))
    tid = np.asarray(inputs["train_ids"]).astype(np.int64)
    n2e = np.asarray(inputs["node2edge_idx"]).astype(np.int64)
    adj = np.asarray(inputs["edge_node_adj"]).astype(np.int64)

    def arr(k):
        return np.asarray(inputs[k], np.float32)

    w32 = np.zeros((128, 2 * NMP * 2 * 256), np.float32)
    w64 = np.zeros((64, NMP * 2 * 256), np.float32)
    w256 = np.zeros((128, 2 * NMP * 2 * 256), np.float32)  # cast to bf16 below
    for b in range(NMP):
        prep_w = arr("edge_prep_w")[b]
        for i, wmat in enumerate([prep_w @ _wflat(arr("edge_wq_l0")[b]),
                                  prep_w @ _wflat(arr("node_wk_l0")[b])]):
            wstk = np.tile(wmat, (8, 1))          # [256, 256]
            for kb in range(2):
                base = ((kb * NMP + b) * 2 + i) * 256
                w32[:, base:base + 256] = wstk[kb * 128:(kb + 1) * 128]
        w64[:, (b * 2 + 0) * 256:(b * 2 + 1) * 256] = arr("W_prep1") @ _wflat(arr("edge_wk_l0")[b])
        w64[:, (b * 2 + 1) * 256:(b * 2 + 2) * 256] = arr("W_prep0") @ _wflat(arr("node_wq_l0")[b])
        wnk1 = _wflat(arr("node_wk_l1")[b])
        wq1 = _wflat(arr("node_wq_l1")[b])
        for kb in range(2):
            w256[:, ((kb * NMP + b) * 2 + 0) * 256:((kb * NMP + b) * 2 + 1) * 256] = \
                wnk1[kb * 128:(kb + 1) * 128]
            w256[:, ((kb * NMP + b) * 2 + 1) * 256:((kb * NMP + b) * 2 + 2) * 256] = \
                wq1[kb * 128:(kb + 1) * 128]
    import ml_dtypes
    wg = np.ascontiguousarray(arr("Wg").reshape(4, 128, 128).transpose(1, 0, 2)
                              .reshape(128, 4 * 128)).astype(ml_dtypes.bfloat16)
    vg = arr("vg").reshape(128, 1)
    wfc = np.ascontiguousarray(arr("W_fc").reshape(4, 128, NCLS).transpose(1, 0, 2)
                               .reshape(128, 4 * NCLS)).astype(ml_dtypes.bfloat16)
    bfc = arr("b_fc").reshape(NCLS, 1)
    hmb, exb, hm, exc = build_host_consts(rpc)

    w256 = w256.astype(ml_dtypes.bfloat16)
    shared = dict(feats=feats, edge_emb=emb, w32=w32, w64=w64, w256=w256,
                  wg=wg, vg=vg, wfc=wfc, bfc=bfc, hmb=hmb, exb=exb, hm=hm, ex=exc,
                  ones1=np.ones((1, 128), np.float32),
                  iota128=np.tile(np.arange(256) // 32, (128, 1)).astype(np.float32))

    slots = rpc * S
    nblk = slots // 128
    sblk = nblk // NSEG
    in_maps = []
    for c in range(n_cores):
        rows = tid[c * rpc:(c + 1) * rpc]
        embidx = np.zeros((128, NMP * (slots // 16)), np.int16)
        subv = np.zeros((128, NMP * nblk), np.float32)
        n0w = np.zeros((128, NMP * (slots // 16)), np.int16)
        n1w = np.zeros((128, NMP * (slots // 16)), np.int16)
        for b in range(NMP):
            eidx = n2e[b][rows].reshape(-1)
            a = adj[b][eidx]
            subv[:, b * nblk:(b + 1) * nblk] = (eidx & 7).reshape(nblk, 128).T
            for seg in range(NSEG):
                lo = seg * sblk * 128
                hi = (seg + 1) * sblk * 128
                sl = slice(b * (slots // 16) + seg * sblk * 8,
                           b * (slots // 16) + (seg + 1) * sblk * 8)
                embidx[:, sl] = _wrap16_rep(eidx[lo:hi] >> 3)
                n0w[:, sl] = _wrap16_rep(a[lo:hi, 0])
                n1w[:, sl] = _wrap16_rep(a[lo:hi, 1])
        tidw = _wrap16_rep(rows)
        m = dict(shared)
        m.update(embidx=embidx, sub=subv, n0idx=n0w, n1idx=n1w, tididx=tidw)
        in_maps.append(m)
    return in_maps


_CACHED_NC = None


def kernel(**inputs):
    global _CACHED_NC
    if _CACHED_NC is None:
        _CACHED_NC = build_program(RPC)
    nc = _CACHED_NC
    in_maps = host_prep(inputs, RPC, N_CORES)
    from concourse.bass_utils import run_bass_kernel_spmd
    res = run_bass_kernel_spmd(nc, in_maps, core_ids=list(range(N_CORES)))
    logits = np.concatenate([res.results[c]["logits_o"] for c in range(N_CORES)], axis=0)
    gate = np.concatenate([res.results[c]["gate_o"] for c in range(N_CORES)], axis=1)
    return logits.astype(np.float32), gate.astype(np.float32)


# revision 18
# speedup vs baseline: 1.0024x; 1.0024x over previous
"""Trainium2 Bass kernel for nn_BipartiteGCN (gnn_message_passing).

Strategy (derived from the reference dataflow):
  * The final output only consumes concat(skip)[train_ids] (B=4096 rows) and
    the layer-1 edge update is never used, so the computation collapses to
    per-(train-row, slot) work on the 16 incident edges of each sampled node:
    the edge-l0 update for exactly those B*16 slots per branch, then the two
    node updates restricted to the sampled rows.
  * Shard the 4096 train rows across 8 NeuronCores (512 rows/core); zero
    inter-core communication. Feature gathers run on-device (dma_gather /
    indirect DMA) from full replicated feats / edge_emb tables; the host only
    composes integer index arrays and combines weight matrices.
  * On-chip layout is feature-major: gathered rows are PE-transposed once,
    projections are fp32r tensor-engine matmuls, attention logit reductions
    are head-mask matmuls, softmax over s=16 runs packed on [64,512] tiles,
    and aggregation uses strided vector reduces.
"""

import numpy as np

import concourse.bass as bass
import concourse.bacc as bacc
import concourse.mybir as mybir
import concourse.tile as tile
from concourse.masks import make_identity

F32 = mybir.dt.float32
F32R = mybir.dt.float32r
BF16 = mybir.dt.bfloat16
I16 = mybir.dt.int16
I32 = mybir.dt.int32
AL = mybir.AluOpType
ACTF = mybir.ActivationFunctionType

# problem dims
N_NODES, S, E_EDGES, NMP, H, O = 20000, 16, 160000, 2, 4, 64
D_FEAT, E_DIM, PREP, NCLS, B = 64, 32, 128, 8, 4096
D1 = H * O  # 256

N_CORES = 8
RPC = B // N_CORES          # 512 train rows per core
NSEG = 8                    # gather segments per branch


def _wflat(w):
    return np.ascontiguousarray(np.transpose(w, (1, 0, 2)).reshape(w.shape[1], H * O))


def _wrap16_rep(idx):
    """dma_gather index layout: [128, n/16] int16, 16-row wrapped, replicated x8."""
    n = idx.shape[0]
    w = idx.reshape(n // 16, 16).T.astype(np.int16)
    return np.tile(w, (8, 1)).copy()


def build_host_consts(rpc):
    import ml_dtypes
    nch = rpc // 32
    # edge (per-chunk) base masks
    hmb = np.zeros((128, 2 * 4), np.float32)
    exb = np.zeros((4, 2 * 128), np.float32)
    for blk in range(2):
        for dd in range(128):
            h = (blk * 128 + dd) // 64
            hmb[dd, blk * 4 + h] = 1.0
            exb[h, blk * 128 + dd] = 1.0
    # node packed masks
    hm = np.zeros((128, 2, nch, 4 * nch), np.float32)
    ex = np.zeros((4 * nch, 2, nch, 128), np.float32)
    for blk in range(2):
        for dd in range(128):
            h = (blk * 128 + dd) // 64
            for c in range(nch):
                hm[dd, blk, c, 4 * c + h] = 1.0
                ex[4 * c + h, blk, c, dd] = 1.0
    bf = ml_dtypes.bfloat16
    return (hmb.astype(bf), exb.astype(bf),
            hm.reshape(128, -1).astype(bf), ex.reshape(4 * nch, -1).astype(bf))


def build_program(rpc=RPC):
    nch = rpc // 32            # 512-slot chunks per branch
    slots = rpc * S
    nblk = slots // 128
    rb = rpc // 128

    nc = bacc.Bacc("TRN2", target_bir_lowering=False, debug=False,
                   enable_asserts=False, num_devices=N_CORES)

    d = {}
    def din(name, shape, dt):
        d[name] = nc.dram_tensor(name, shape, dt, kind="ExternalInput")
        return d[name]

    din("feats", (N_NODES, D_FEAT), F32)
    din("edge_emb", (NMP, E_EDGES, E_DIM), F32)
    din("w32", (128, 2 * NMP * 2 * 256), F32R)
    din("w64", (64, NMP * 2 * 256), F32R)
    din("w256", (128, 2 * NMP * 2 * 256), BF16)
    din("wg", (128, 4 * 128), BF16)
    din("vg", (128, 1), F32R)
    din("ones1", (1, 128), F32R)
    din("wfc", (128, 4 * NCLS), BF16)
    din("bfc", (NCLS, 1), F32)
    din("hmb", (128, 2 * 4), BF16)
    din("exb", (4, 2 * 128), BF16)
    din("hm", (128, 2 * nch * 4 * nch), BF16)
    din("ex", (4 * nch, 2 * nch * 128), BF16)
    din("embidx", (128, NMP * (slots // 16)), I16)
    din("sub", (128, NMP * nblk), F32)
    din("iota128", (128, 256), F32)
    din("n0idx", (128, NMP * (slots // 16)), I16)
    din("n1idx", (128, NMP * (slots // 16)), I16)
    din("tididx", (128, rpc // 16), I16)

    logits_o = nc.dram_tensor("logits_o", (rpc, NCLS), F32, kind="ExternalOutput")
    gate_o = nc.dram_tensor("gate_o", (NMP, rpc), F32, kind="ExternalOutput")

    with tile.TileContext(nc) as tc:
        _emit(nc, tc, d, logits_o, gate_o, rpc, nch, slots, nblk, rb)

    nc.compile()
    return nc


def _emit(nc, tc, d, logits_o, gate_o, rpc, nch, slots, nblk, rb):
    from contextlib import ExitStack
    ctx = ExitStack()
    cst = ctx.enter_context(tc.tile_pool(name="cst", bufs=1))
    big = ctx.enter_context(tc.tile_pool(name="big", bufs=1))
    gat = ctx.enter_context(tc.tile_pool(name="gat", bufs=1))
    tloc = ctx.enter_context(tc.tile_pool(name="tloc", bufs=2))
    wk = ctx.enter_context(tc.tile_pool(name="wk", bufs=1))
    ps = ctx.enter_context(tc.tile_pool(name="ps", bufs=2, space="PSUM"))
    drp = ctx.enter_context(tc.tile_pool(name="drp", bufs=1, space="DRAM"))

    ident = cst.tile([128, 128], F32)
    make_identity(nc, ident[:])


    def ctile(name, shape, dt):
        t = cst.tile(shape, dt, tag=name)
        nc.sync.dma_start(t[:], d[name][:])
        return t

    w32 = ctile("w32", [128, 2 * NMP * 2 * 256], F32R)
    w64 = ctile("w64", [64, NMP * 2 * 256], F32R)
    w256 = ctile("w256", [128, 2 * NMP * 2 * 256], BF16)
    wg = ctile("wg", [128, 4 * 128], BF16)
    vg = ctile("vg", [128, 1], F32R)
    ones1 = ctile("ones1", [1, 128], F32R)
    wfc = ctile("wfc", [128, 4 * NCLS], BF16)
    bfc = ctile("bfc", [NCLS, 1], F32)
    hmb = ctile("hmb", [128, 2 * 4], BF16)
    exb = ctile("exb", [4, 2 * 128], BF16)
    hm = ctile("hm", [128, 2 * nch * 4 * nch], BF16)
    ex = ctile("ex", [4 * nch, 2 * nch * 128], BF16)
    embidx = ctile("embidx", [128, NMP * (slots // 16)], I16)
    subt = ctile("sub", [128, NMP * nblk], F32)
    iota128 = ctile("iota128", [128, 256], F32)
    n0i = ctile("n0idx", [128, NMP * (slots // 16)], I16)
    n1i = ctile("n1idx", [128, NMP * (slots // 16)], I16)
    tidi = ctile("tididx", [128, rpc // 16], I16)

    def r32(x):
        return x.bitcast(F32R)

    def w32s(kb, b, i, j):
        base = ((kb * NMP + b) * 2 + i) * 256
        return w32[:, base + j * 128: base + (j + 1) * 128]

    def w64s(b, i, j):
        base = (b * 2 + i) * 256
        return w64[:, base + j * 128: base + (j + 1) * 128]

    def w256s(kb, b, i, j):
        base = ((kb * NMP + b) * 2 + i) * 256
        return w256[:, base + j * 128: base + (j + 1) * 128]

    def hms(blk, c):
        return hm[:, (blk * nch + c) * 4 * nch:(blk * nch + c + 1) * 4 * nch]

    def exs(blk, c):
        return ex[:, (blk * nch + c) * 128:(blk * nch + c + 1) * 128]

    NP = 4 * nch
    CPS = nch // NSEG          # chunks per segment

    # ---- FT gather + transpose ----
    ft = wk.tile([128, rb, 64], F32, tag="ft")
    nc.gpsimd.dma_gather(out_ap=ft[:], in_ap=d["feats"][:], idxs_ap=tidi[:],
                         num_idxs=rpc, num_idxs_reg=rpc, elem_size=64)
    ftt = cst.tile([64, rpc], F32R)
    pT = ps.tile([64, rpc], F32, tag="pa", bufs=4)
    for k in range(rb):
        nc.tensor.matmul(pT[:, k * 128:(k + 1) * 128], ft[:, k, :], ident[:],
                         is_transpose=True, start=(k == 0), stop=(k == rb - 1))
    nc.scalar.copy(ftt[:], pT[:])

    f12 = [[None, None], [None, None]]

    for b in range(NMP):
        k0d = drp.tile([128, 2, slots], BF16, tag=f"k0d{b}")
        k1d = drp.tile([128, 2, slots], BF16, tag=f"k1d{b}")

        for seg in range(NSEG):
            sblk = nblk // NSEG          # blocks per segment
            so = seg * sblk
            # ---- segment gathers ----
            oct = gat.tile([128, sblk, 256], F32, tag="oct", bufs=2)
            ioff = b * (slots // 16) + so * 8
            embv = d["edge_emb"][b].rearrange("(u k) d -> u (k d)", k=8)
            nc.gpsimd.dma_gather(out_ap=oct[:], in_ap=embv,
                                 idxs_ap=embidx[:, ioff: ioff + sblk * 8],
                                 num_idxs=sblk * 128, num_idxs_reg=sblk * 128,
                                 elem_size=256)
            f0 = gat.tile([128, sblk, 64], F32, tag="f0", bufs=2)
            nc.gpsimd.dma_gather(out_ap=f0[:], in_ap=d["feats"][:],
                                 idxs_ap=n0i[:, ioff: ioff + sblk * 8],
                                 num_idxs=sblk * 128, num_idxs_reg=sblk * 128,
                                 elem_size=64)
            f1 = gat.tile([128, sblk, 64], F32, tag="f1", bufs=2)
            nc.gpsimd.dma_gather(out_ap=f1[:], in_ap=d["feats"][:],
                                 idxs_ap=n1i[:, ioff: ioff + sblk * 8],
                                 num_idxs=sblk * 128, num_idxs_reg=sblk * 128,
                                 elem_size=64)
            df = gat.tile([128, sblk, 64], F32, tag="df", bufs=2)
            nc.vector.tensor_tensor(df[:], f0[:], f1[:], op=AL.subtract)

            for ci in range(CPS):
                gc = seg * CPS + ci
                cs = slice(gc * 512, (gc + 1) * 512)
                # ---- sub-row select: memb = (iota==sub) * octet ----
                mmk = wk.tile([128, 4, 256], F32, tag="mmk", bufs=2)
                for k in range(4):
                    blki = ci * 4 + k
                    nc.vector.scalar_tensor_tensor(
                        mmk[:, k, :], iota128[:],
                        subt[:, b * nblk + so + blki: b * nblk + so + blki + 1],
                        oct[:, blki, :], op0=AL.is_equal, op1=AL.mult)
                # ---- transposes ----
                pmT = ps.tile([128, 2, 512], F32, tag="pb", bufs=2)
                p64a = ps.tile([64, 512], F32, tag="pa", bufs=4)
                p64b = ps.tile([64, 512], F32, tag="pa", bufs=4)
                for k in range(4):
                    blki = ci * 4 + k
                    for h in range(2):
                        nc.tensor.matmul(pmT[:, h, k * 128:(k + 1) * 128],
                                         mmk[:, k, h * 128:(h + 1) * 128],
                                         ident[:], is_transpose=True,
                                         start=(k == 0), stop=(k == 3))
                    nc.tensor.matmul(p64a[:, k * 128:(k + 1) * 128], f1[:, blki, :],
                                     ident[:], is_transpose=True,
                                     start=(k == 0), stop=(k == 3))
                    nc.tensor.matmul(p64b[:, k * 128:(k + 1) * 128], df[:, blki, :],
                                     ident[:], is_transpose=True,
                                     start=(k == 0), stop=(k == 3))
                mT = tloc.tile([128, 2, 512], F32R, tag="mT")
                nc.scalar.copy(mT[:], pmT[:])
                f1t = tloc.tile([64, 512], F32R, tag="f1t")
                nc.scalar.copy(f1t[:], p64a[:])
                dft = tloc.tile([64, 512], F32R, tag="dft")
                nc.scalar.copy(dft[:], p64b[:])

                # ---- projections ----
                qep = ps.tile([128, 2, 512], F32, tag="pb", bufs=2)
                dtp = ps.tile([128, 2, 512], F32, tag="pb", bufs=2)
                for j in range(2):
                    for kb in range(2):
                        nc.tensor.matmul(qep[:, j, :], w32s(kb, b, 0, j),
                                         mT[:, kb, :], start=(kb == 0), stop=(kb == 1))
                    nc.tensor.matmul(dtp[:, j, :], w64s(b, 0, j), dft[:],
                                     start=True, stop=True)
                dts = wk.tile([128, 2, 512], F32, tag="wf", bufs=4)
                nc.scalar.copy(dts[:], dtp[:])
                pp = wk.tile([128, 2, 512], BF16, tag="wh", bufs=3)
                nc.vector.tensor_tensor(pp[:], qep[:], dts[:], op=AL.mult)
                ldc = ps.tile([4, 512], F32, tag="pa", bufs=4)
                for blk in range(2):
                    nc.tensor.matmul(ldc[:], hmb[:, blk * 4:(blk + 1) * 4],
                                     pp[:, blk, :], start=(blk == 0), stop=(blk == 1))
                a0 = wk.tile([4, 512], BF16, tag="a0", bufs=1)
                nc.scalar.activation(a0[:], ldc[:], ACTF.Sigmoid, scale=0.125)

                ktp = ps.tile([128, 2, 512], F32, tag="pb", bufs=2)
                pre0 = ps.tile([128, 2, 512], F32, tag="pb", bufs=2)
                for j in range(2):
                    for kb in range(2):
                        nc.tensor.matmul(ktp[:, j, :], w32s(kb, b, 1, j),
                                         mT[:, kb, :], start=(kb == 0), stop=(kb == 1))
                        nc.tensor.matmul(pre0[:, j, :], w32s(kb, b, 0, j),
                                         mT[:, kb, :], start=(kb == 0), stop=False)
                    nc.tensor.matmul(pre0[:, j, :], w64s(b, 0, j), f1t[:],
                                     start=False, stop=True)
                k0c = wk.tile([128, 2, 512], BF16, tag="wh", bufs=3)
                nc.scalar.copy(k0c[:], ktp[:])
                nc.sync.dma_start(k0d[:, :, cs], k0c[:])
                aep = ps.tile([128, 2, 512], F32, tag="pb", bufs=2)
                for blk in range(2):
                    nc.tensor.matmul(aep[:, blk, :], exb[:, blk * 128:(blk + 1) * 128],
                                     a0[:], start=True, stop=True)
                tt = wk.tile([128, 2, 512], F32, tag="wf", bufs=4)
                nc.vector.tensor_tensor(tt[:], aep[:], dts[:], op=AL.mult)
                pre = wk.tile([128, 2, 512], F32, tag="wf", bufs=4)
                nc.vector.tensor_tensor(pre[:], tt[:], pre0[:], op=AL.add)
                npre = wk.tile([128, 2, 512], F32, tag="wf", bufs=4)
                nc.vector.tensor_scalar_min(npre[:], pre[:], 0.0)
                epre = wk.tile([128, 2, 512], F32, tag="wf", bufs=4)
                nc.scalar.activation(epre[:], npre[:], ACTF.Exp)
                em1 = wk.tile([128, 2, 512], F32, tag="wf", bufs=4)
                nc.vector.tensor_scalar_add(em1[:], epre[:], -1.0)
                e1 = wk.tile([128, 2, 512], BF16, tag="wh", bufs=3)
                nc.vector.scalar_tensor_tensor(e1[:], pre[:], 0.0, em1[:],
                                               op0=AL.max, op1=AL.add)
                k1p = ps.tile([128, 2, 512], F32, tag="pb", bufs=2)
                for j in range(2):
                    for kb in range(2):
                        nc.tensor.matmul(k1p[:, j, :], w256s(kb, b, 0, j),
                                         e1[:, kb, :],
                                         start=(kb == 0), stop=(kb == 1))
                k1c = wk.tile([128, 2, 512], BF16, tag="wh", bufs=3)
                nc.scalar.copy(k1c[:], k1p[:])
                nc.sync.dma_start(k1d[:, :, cs], k1c[:])

        # ================= node layers =================
        GRP = 1024                       # slots per batched vector group
        NGR = slots // GRP
        for l in range(2):
            qp = ps.tile([128, 2, rpc], F32, tag="pb", bufs=2)
            if l == 0:
                for j in range(2):
                    nc.tensor.matmul(qp[:, j, :], w64s(b, 1, j), ftt[:],
                                     start=True, stop=True)
            else:
                for j in range(2):
                    for kb in range(2):
                        nc.tensor.matmul(qp[:, j, :], w256s(kb, b, 1, j),
                                         f12[b][0][:, kb, :],
                                         start=(kb == 0), stop=(kb == 1))
            qf = wk.tile([128, 2, rpc], F32, tag="qf", bufs=1)
            nc.scalar.copy(qf[:], qp[:])
            qh = wk.tile([128, 2, rpc], BF16, tag="qh", bufs=1)
            nc.vector.tensor_copy(qh[:], qf[:])

            kd = k0d if l == 0 else k1d
            def kt_group(g):
                gs = slice(g * GRP, (g + 1) * GRP)
                kt = wk.tile([128, 2, GRP], BF16, tag="ktc", bufs=3)
                nc.sync.dma_start(kt[:], kd[:, :, gs])
                return kt[:]

            ln = ps.tile([NP, 512], F32, tag="pa", bufs=4)
            for g in range(NGR):
                ktv = kt_group(g)
                rw = GRP // 16
                qb = qh[:, :, g * rw:(g + 1) * rw].unsqueeze(3).to_broadcast(
                    [128, 2, rw, 16])
                pr = wk.tile([128, 2, GRP], BF16, tag="pp2", bufs=2)
                nc.vector.tensor_tensor(
                    pr[:].rearrange("p b (r s) -> p b r s", s=16), qb,
                    ktv.rearrange("p b (r s) -> p b r s", s=16), op=AL.mult)
                for cc in range(GRP // 512):
                    c = g * (GRP // 512) + cc
                    for blk in range(2):
                        nc.tensor.matmul(ln[:], hms(blk, c),
                                         pr[:, blk, cc * 512:(cc + 1) * 512],
                                         start=(c == 0 and blk == 0),
                                         stop=(c == nch - 1 and blk == 1))

            lnv = ln[:].rearrange("p (r s) -> p r s", s=16)
            mx = wk.tile([NP, 32], F32, tag="mx", bufs=2)
            nc.vector.tensor_reduce(mx[:], lnv, axis=mybir.AxisListType.X, op=AL.max)
            sub = wk.tile([NP, 512], F32, tag="wf", bufs=4)
            nc.vector.tensor_tensor(sub[:].rearrange("p (r s) -> p r s", s=16), lnv,
                                    mx[:].unsqueeze(2).to_broadcast([NP, 32, 16]),
                                    op=AL.subtract)
            esub = wk.tile([NP, 512], F32, tag="wf", bufs=4)
            nc.scalar.activation(esub[:], sub[:], ACTF.Exp, scale=0.125)
            zs = wk.tile([NP, 32], F32, tag="zs", bufs=2)
            nc.vector.tensor_reduce(zs[:], esub[:].rearrange("p (r s) -> p r s", s=16),
                                    axis=mybir.AxisListType.X, op=AL.add)
            zr = wk.tile([NP, 32], F32, tag="zr", bufs=2)
            nc.vector.reciprocal(zr[:], zs[:])
            pn = wk.tile([NP, 512], BF16, tag="pn", bufs=2)
            nc.vector.tensor_tensor(pn[:].rearrange("p (r s) -> p r s", s=16),
                                    esub[:].rearrange("p (r s) -> p r s", s=16),
                                    zr[:].unsqueeze(2).to_broadcast([NP, 32, 16]),
                                    op=AL.mult)

            agg = wk.tile([128, 2, rpc], F32, tag="agg", bufs=1)
            for g in range(NGR):
                aeh = wk.tile([128, 2, GRP], BF16, tag="aeh2", bufs=2)
                for cc in range(GRP // 512):
                    c = g * (GRP // 512) + cc
                    aep = ps.tile([128, 2, 512], F32, tag="pb", bufs=2)
                    for blk in range(2):
                        nc.tensor.matmul(aep[:, blk, :], exs(blk, c), pn[:],
                                         start=True, stop=True)
                    nc.scalar.copy(aeh[:, :, cc * 512:(cc + 1) * 512], aep[:])
                ak = wk.tile([128, 2, GRP], BF16, tag="pp2", bufs=2)
                nc.vector.tensor_tensor(ak[:], aeh[:], kt_group(g), op=AL.mult)
                rw = GRP // 16
                nc.vector.tensor_reduce(agg[:, :, g * rw:(g + 1) * rw],
                                        ak[:].rearrange("p b (r s) -> p b r s", s=16),
                                        axis=mybir.AxisListType.X, op=AL.add)

            prn = wk.tile([128, 2, rpc], F32, tag="wf", bufs=4)
            nc.vector.tensor_tensor(prn[:], qf[:], agg[:], op=AL.add)
            npre = wk.tile([128, 2, rpc], F32, tag="wf", bufs=4)
            nc.vector.tensor_scalar_min(npre[:], prn[:], 0.0)
            epre = wk.tile([128, 2, rpc], F32, tag="wf", bufs=4)
            nc.scalar.activation(epre[:], npre[:], ACTF.Exp)
            em1 = wk.tile([128, 2, rpc], F32, tag="wf", bufs=4)
            nc.vector.tensor_scalar_add(em1[:], epre[:], -1.0)
            fl = big.tile([128, 2, rpc], BF16, tag=f"f12_{b}_{l}")
            nc.vector.scalar_tensor_tensor(fl[:], prn[:], 0.0, em1[:],
                                           op0=AL.max, op1=AL.add)
            f12[b][l] = fl

    # ================= finale =================
    gsb = []
    for b in range(NMP):
        tp = ps.tile([128, rpc], F32, tag="pb", bufs=2)
        for kb in range(4):
            l, jb = kb // 2, kb % 2
            nc.tensor.matmul(tp[:], wg[:, kb * 128:(kb + 1) * 128],
                             f12[b][l][:, jb, :], start=(kb == 0), stop=(kb == 3))
        th = wk.tile([128, rpc], F32R, tag="wf", bufs=4)
        nc.scalar.activation(th[:], tp[:], ACTF.Tanh)
        gp = ps.tile([1, rpc], F32, tag="pa", bufs=4)
        nc.tensor.matmul(gp[:], vg[:], th[:], start=True, stop=True)
        g = wk.tile([1, rpc], F32, tag=f"g{b}")
        nc.scalar.copy(g[:], gp[:])
        gsb.append(g)

    gd = wk.tile([1, rpc], F32, tag="gd")
    nc.vector.tensor_tensor(gd[:], gsb[0][:], gsb[1][:], op=AL.subtract)
    gate0 = wk.tile([1, rpc], F32, tag="gate0")
    nc.scalar.activation(gate0[:], gd[:], ACTF.Sigmoid)
    gate1 = wk.tile([1, rpc], F32, tag="gate1")
    nc.vector.tensor_scalar(gate1[:], gate0[:], -1.0, 1.0, op0=AL.mult, op1=AL.add)
    nc.sync.dma_start(gate_o[0:1, :], gate0[:])
    nc.sync.dma_start(gate_o[1:2, :], gate1[:])

    ge = []
    g0r = wk.tile([1, rpc], F32R, tag="g0r")
    nc.scalar.copy(g0r[:], gate0[:])
    g1r = wk.tile([1, rpc], F32R, tag="g1r")
    nc.scalar.copy(g1r[:], gate1[:])
    for b in range(NMP):
        gep = ps.tile([128, rpc], F32, tag="pb", bufs=2)
        nc.tensor.matmul(gep[:], ones1[:],
                         g0r[:] if b == 0 else g1r[:], start=True, stop=True)
        gs = wk.tile([128, rpc], BF16, tag=f"ge{b}")
        nc.scalar.copy(gs[:], gep[:])
        ge.append(gs)
    pooled = [None, None]
    for l in range(2):
        t0 = wk.tile([128, 2, rpc], BF16, tag="wh", bufs=3)
        nc.vector.tensor_tensor(t0[:], ge[0][:].unsqueeze(1).to_broadcast([128, 2, rpc]),
                                f12[0][l][:], op=AL.mult)
        t1 = wk.tile([128, 2, rpc], BF16, tag="wh", bufs=3)
        nc.vector.tensor_tensor(t1[:], ge[1][:].unsqueeze(1).to_broadcast([128, 2, rpc]),
                                f12[1][l][:], op=AL.mult)
        pl = wk.tile([128, 2, rpc], BF16, tag=f"pool{l}")
        nc.vector.tensor_tensor(pl[:], t0[:], t1[:], op=AL.add)
        pooled[l] = pl

    lg = ps.tile([NCLS, rpc], F32, tag="pa", bufs=4)
    for kb in range(4):
        l, jb = kb // 2, kb % 2
        nc.tensor.matmul(lg[:], wfc[:, kb * NCLS:(kb + 1) * NCLS],
                         pooled[l][:, jb, :], start=(kb == 0), stop=(kb == 3))
    lgb = wk.tile([NCLS, rpc], F32, tag="lgb")
    nc.vector.tensor_scalar_add(lgb[:], lg[:], bfc[:, 0:1])

    rbk = rpc // 128
    lgT = wk.tile([128, rbk * NCLS], F32, tag="lgT")
    plg = ps.tile([128, rbk * NCLS], F32, tag="pa", bufs=4)
    for k in range(rbk):
        nc.tensor.matmul(plg[:, k * NCLS:(k + 1) * NCLS], lgb[:, k * 128:(k + 1) * 128],
                         ident[0:NCLS, 0:NCLS], is_transpose=True,
                         start=(k == 0), stop=(k == rbk - 1))
    nc.scalar.copy(lgT[:], plg[:])
    nc.sync.dma_start(logits_o[:].rearrange("(k p) c -> p k c", p=128),
                      lgT[:].rearrange("p (k c) -> p k c", c=NCLS))

    ctx.close()


def host_prep(inputs, rpc=RPC, n_cores=N_CORES):
    feats = np.ascontiguousarray(np.asarray(inputs["feats"], np.float32))
    emb = np.ascontiguousarray(np.asarray(inputs["edge_emb"], np.float32))
    tid = np.asarray(inputs["train_ids"]).astype(np.int64)
    n2e = np.asarray(inputs["node2edge_idx"]).astype(np.int64)
    adj = np.asarray(inputs["edge_node_adj"]).astype(np.int64)

    def arr(k):
        return np.asarray(inputs[k], np.float32)

    w32 = np.zeros((128, 2 * NMP * 2 * 256), np.float32)
    w64 = np.zeros((64, NMP * 2 * 256), np.float32)
    w256 = np.zeros((128, 2 * NMP * 2 * 256), np.float32)  # cast to bf16 below
    for b in range(NMP):
        prep_w = arr("edge_prep_w")[b]
        for i, wmat in enumerate([prep_w @ _wflat(arr("edge_wq_l0")[b]),
                                  prep_w @ _wflat(arr("node_wk_l0")[b])]):
            wstk = np.tile(wmat, (8, 1))          # [256, 256]
            for kb in range(2):
                base = ((kb * NMP + b) * 2 + i) * 256
                w32[:, base:base + 256] = wstk[kb * 128:(kb + 1) * 128]
        w64[:, (b * 2 + 0) * 256:(b * 2 + 1) * 256] = arr("W_prep1") @ _wflat(arr("edge_wk_l0")[b])
        w64[:, (b * 2 + 1) * 256:(b * 2 + 2) * 256] = arr("W_prep0") @ _wflat(arr("node_wq_l0")[b])
        wnk1 = _wflat(arr("node_wk_l1")[b])
        wq1 = _wflat(arr("node_wq_l1")[b])
        for kb in range(2):
            w256[:, ((kb * NMP + b) * 2 + 0) * 256:((kb * NMP + b) * 2 + 1) * 256] = \
                wnk1[kb * 128:(kb + 1) * 128]
            w256[:, ((kb * NMP + b) * 2 + 1) * 256:((kb * NMP + b) * 2 + 2) * 256] = \
                wq1[kb * 128:(kb + 1) * 128]
    import ml_dtypes
    wg = np.ascontiguousarray(arr("Wg").reshape(4, 128, 128).transpose(1, 0, 2)
                              .reshape(128, 4 * 128)).astype(ml_dtypes.bfloat16)
    vg = arr("vg").reshape(128, 1)
    wfc = np.ascontiguousarray(arr("W_fc").reshape(4, 128, NCLS).transpose(1, 0, 2)
                               .reshape(128, 4 * NCLS)).astype(ml_dtypes.bfloat16)
    bfc = arr("b_fc").reshape(NCLS, 1)
    hmb, exb, hm, exc = build_host_consts(rpc)

    w256 = w256.astype(ml_dtypes.bfloat16)
    shared = dict(feats=feats, edge_emb=emb, w32=w32, w64=w64, w256=w256,
                  wg=wg, vg=vg, wfc=wfc, bfc=bfc, hmb=hmb, exb=exb, hm=hm, ex=exc,
                  ones1=np.ones((1, 128), np.float32),
                  iota128=np.tile(np.arange(256) // 32, (128, 1)).astype(np.float32))

    slots = rpc * S
    nblk = slots // 128
    sblk = nblk // NSEG
    in_maps = []
    for c in range(n_cores):
        rows = tid[c * rpc:(c + 1) * rpc]
        embidx = np.zeros((128, NMP * (slots // 16)), np.int16)
        subv = np.zeros((128, NMP * nblk), np.float32)
        n0w = np.zeros((128, NMP * (slots // 16)), np.int16)
        n1w = np.zeros((128, NMP * (slots // 16)), np.int16)
        for b in range(NMP):
            eidx = n2e[b][rows].reshape(-1)
            a = adj[b][eidx]
            subv[:, b * nblk:(b + 1) * nblk] = (eidx & 7).reshape(nblk, 128).T
            for seg in range(NSEG):
                lo = seg * sblk * 128
                hi = (seg + 1) * sblk * 128
                sl = slice(b * (slots // 16) + seg * sblk * 8,
                           b * (slots // 16) + (seg + 1) * sblk * 8)
                embidx[:, sl] = _wrap16_rep(eidx[lo:hi] >> 3)
                n0w[:, sl] = _wrap16_rep(a[lo:hi, 0])
                n1w[:, sl] = _wrap16_rep(a[lo:hi, 1])
        tidw = _wrap16_rep(rows)
        m = dict(shared)
        m.update(embidx=embidx, sub=subv, n0idx=n0w, n1idx=n1w, tididx=tidw)
        in_maps.append(m)
    return in_maps


_CACHED_NC = None


def kernel(**inputs):
    global _CACHED_NC
    if _CACHED_NC is None:
        _CACHED_NC = build_program(RPC)
    nc = _CACHED_NC
    in_maps = host_prep(inputs, RPC, N_CORES)
    from concourse.bass_utils import run_bass_kernel_spmd
    res = run_bass_kernel_spmd(nc, in_maps, core_ids=list(range(N_CORES)))
    logits = np.concatenate([res.results[c]["logits_o"] for c in range(N_CORES)], axis=0)
    gate = np.concatenate([res.results[c]["gate_o"] for c in range(N_CORES)], axis=1)
    return logits.astype(np.float32), gate.astype(np.float32)


# revision 19
# speedup vs baseline: 1.1331x; 1.1304x over previous
"""Trainium2 Bass kernel for nn_BipartiteGCN (gnn_message_passing).

Strategy (derived from the reference dataflow):
  * The final output only consumes concat(skip)[train_ids] (B=4096 rows) and
    the layer-1 edge update is never used, so the computation collapses to
    per-(train-row, slot) work on the 16 incident edges of each sampled node:
    the edge-l0 update for exactly those B*16 slots per branch, then the two
    node updates restricted to the sampled rows.
  * Shard the 4096 train rows across 8 NeuronCores (512 rows/core); zero
    inter-core communication. Feature gathers run on-device (dma_gather /
    indirect DMA) from full replicated feats / edge_emb tables; the host only
    composes integer index arrays and combines weight matrices.
  * On-chip layout is feature-major: gathered rows are PE-transposed once,
    projections are fp32r tensor-engine matmuls, attention logit reductions
    are head-mask matmuls, softmax over s=16 runs packed on [64,512] tiles,
    and aggregation uses strided vector reduces.
"""

import numpy as np

import concourse.bass as bass
import concourse.bacc as bacc
import concourse.mybir as mybir
import concourse.tile as tile
from concourse.masks import make_identity

F32 = mybir.dt.float32
F32R = mybir.dt.float32r
BF16 = mybir.dt.bfloat16
I16 = mybir.dt.int16
I32 = mybir.dt.int32
AL = mybir.AluOpType
ACTF = mybir.ActivationFunctionType

# problem dims
N_NODES, S, E_EDGES, NMP, H, O = 20000, 16, 160000, 2, 4, 64
D_FEAT, E_DIM, PREP, NCLS, B = 64, 32, 128, 8, 4096
D1 = H * O  # 256

N_CORES = 8
RPC = B // N_CORES          # 512 train rows per core
NSEG = 8                    # gather segments per branch


def _wflat(w):
    return np.ascontiguousarray(np.transpose(w, (1, 0, 2)).reshape(w.shape[1], H * O))


def _wrap16_rep(idx):
    """dma_gather index layout: [128, n/16] int16, 16-row wrapped, replicated x8."""
    n = idx.shape[0]
    w = idx.reshape(n // 16, 16).T.astype(np.int16)
    return np.tile(w, (8, 1)).copy()


def build_host_consts(rpc):
    import ml_dtypes
    nch = rpc // 32
    # edge (per-chunk) base masks
    hmb = np.zeros((128, 2 * 4), np.float32)
    exb = np.zeros((4, 2 * 128), np.float32)
    for blk in range(2):
        for dd in range(128):
            h = (blk * 128 + dd) // 64
            hmb[dd, blk * 4 + h] = 1.0
            exb[h, blk * 128 + dd] = 1.0
    # node packed masks
    hm = np.zeros((128, 2, nch, 4 * nch), np.float32)
    ex = np.zeros((4 * nch, 2, nch, 128), np.float32)
    for blk in range(2):
        for dd in range(128):
            h = (blk * 128 + dd) // 64
            for c in range(nch):
                hm[dd, blk, c, 4 * c + h] = 1.0
                ex[4 * c + h, blk, c, dd] = 1.0
    bf = ml_dtypes.bfloat16
    return (hmb.astype(bf), exb.astype(bf),
            hm.reshape(128, -1).astype(bf), ex.reshape(4 * nch, -1).astype(bf))


def build_program(rpc=RPC):
    nch = rpc // 32            # 512-slot chunks per branch
    slots = rpc * S
    nblk = slots // 128
    rb = rpc // 128

    nc = bacc.Bacc("TRN2", target_bir_lowering=False, debug=False,
                   enable_asserts=False, num_devices=N_CORES)

    d = {}
    def din(name, shape, dt):
        d[name] = nc.dram_tensor(name, shape, dt, kind="ExternalInput")
        return d[name]

    din("feats", (N_NODES, D_FEAT), F32)
    din("edge_emb", (NMP, E_EDGES, E_DIM), F32)
    din("w32", (128, 2 * NMP * 2 * 256), F32R)
    din("w64", (64, NMP * 2 * 256), F32R)
    din("w256", (128, 2 * NMP * 2 * 256), BF16)
    din("wg", (128, 4 * 128), BF16)
    din("vg", (128, 1), F32R)
    din("ones1", (1, 128), F32R)
    din("wfc", (128, 4 * NCLS), BF16)
    din("bfc", (NCLS, 1), F32)
    din("hmb", (128, 2 * 4), BF16)
    din("exb", (4, 2 * 128), BF16)
    din("hm", (128, 2 * nch * 4 * nch), BF16)
    din("ex", (4 * nch, 2 * nch * 128), BF16)
    din("embidx", (128, NMP * (slots // 16)), I16)
    din("sub", (128, NMP * nblk), F32)
    din("iota128", (128, 256), F32)
    din("n0idx", (128, NMP * (slots // 16)), I16)
    din("n1idx", (128, NMP * (slots // 16)), I16)
    din("tididx", (128, rpc // 16), I16)

    logits_o = nc.dram_tensor("logits_o", (rpc, NCLS), F32, kind="ExternalOutput")
    gate_o = nc.dram_tensor("gate_o", (NMP, rpc), F32, kind="ExternalOutput")

    with tile.TileContext(nc) as tc:
        _emit(nc, tc, d, logits_o, gate_o, rpc, nch, slots, nblk, rb)

    nc.compile()
    return nc


def _emit(nc, tc, d, logits_o, gate_o, rpc, nch, slots, nblk, rb):
    from contextlib import ExitStack
    ctx = ExitStack()
    cst = ctx.enter_context(tc.tile_pool(name="cst", bufs=1))
    big = ctx.enter_context(tc.tile_pool(name="big", bufs=1))
    gat = ctx.enter_context(tc.tile_pool(name="gat", bufs=1))
    tloc = ctx.enter_context(tc.tile_pool(name="tloc", bufs=2))
    wk = ctx.enter_context(tc.tile_pool(name="wk", bufs=1))
    ps = ctx.enter_context(tc.tile_pool(name="ps", bufs=2, space="PSUM"))
    drp = ctx.enter_context(tc.tile_pool(name="drp", bufs=1, space="DRAM"))

    ident = cst.tile([128, 128], F32)
    make_identity(nc, ident[:])


    def ctile(name, shape, dt):
        t = cst.tile(shape, dt, tag=name)
        nc.sync.dma_start(t[:], d[name][:])
        return t

    w32 = ctile("w32", [128, 2 * NMP * 2 * 256], F32R)
    w64 = ctile("w64", [64, NMP * 2 * 256], F32R)
    w256 = ctile("w256", [128, 2 * NMP * 2 * 256], BF16)
    wg = ctile("wg", [128, 4 * 128], BF16)
    vg = ctile("vg", [128, 1], F32R)
    ones1 = ctile("ones1", [1, 128], F32R)
    wfc = ctile("wfc", [128, 4 * NCLS], BF16)
    bfc = ctile("bfc", [NCLS, 1], F32)
    hmb = ctile("hmb", [128, 2 * 4], BF16)
    exb = ctile("exb", [4, 2 * 128], BF16)
    hm = ctile("hm", [128, 2 * nch * 4 * nch], BF16)
    ex = ctile("ex", [4 * nch, 2 * nch * 128], BF16)
    embidx = ctile("embidx", [128, NMP * (slots // 16)], I16)
    subt = ctile("sub", [128, NMP * nblk], F32)
    iota128 = ctile("iota128", [128, 256], F32)
    n0i = ctile("n0idx", [128, NMP * (slots // 16)], I16)
    n1i = ctile("n1idx", [128, NMP * (slots // 16)], I16)
    tidi = ctile("tididx", [128, rpc // 16], I16)

    def r32(x):
        return x.bitcast(F32R)

    def w32s(kb, b, i, j):
        base = ((kb * NMP + b) * 2 + i) * 256
        return w32[:, base + j * 128: base + (j + 1) * 128]

    def w64s(b, i, j):
        base = (b * 2 + i) * 256
        return w64[:, base + j * 128: base + (j + 1) * 128]

    def w256s(kb, b, i, j):
        base = ((kb * NMP + b) * 2 + i) * 256
        return w256[:, base + j * 128: base + (j + 1) * 128]

    def hms(blk, c):
        return hm[:, (blk * nch + c) * 4 * nch:(blk * nch + c + 1) * 4 * nch]

    def exs(blk, c):
        return ex[:, (blk * nch + c) * 128:(blk * nch + c + 1) * 128]

    NP = 4 * nch
    CPS = nch // NSEG          # chunks per segment

    # ---- FT gather + transpose ----
    ft = wk.tile([128, rb, 64], F32, tag="ft")
    nc.gpsimd.dma_gather(out_ap=ft[:], in_ap=d["feats"][:], idxs_ap=tidi[:],
                         num_idxs=rpc, num_idxs_reg=rpc, elem_size=64)
    ftt = cst.tile([64, rpc], F32R)
    pT = ps.tile([64, rpc], F32, tag="pa", bufs=2)
    for k in range(rb):
        nc.tensor.matmul(pT[:, k * 128:(k + 1) * 128], ft[:, k, :], ident[:],
                         is_transpose=True, start=(k == 0), stop=(k == rb - 1))
    nc.scalar.copy(ftt[:], pT[:])

    f12 = [[None, None], [None, None]]

    for b in range(NMP):
        k0d = drp.tile([128, 2, slots], BF16, tag=f"k0d{b}")
        k1d = drp.tile([128, 2, slots], BF16, tag=f"k1d{b}")

        for seg in range(NSEG):
            sblk = nblk // NSEG          # blocks per segment
            so = seg * sblk
            # ---- segment gathers ----
            oct = gat.tile([128, sblk, 256], F32, tag="oct", bufs=2)
            ioff = b * (slots // 16) + so * 8
            embv = d["edge_emb"][b].rearrange("(u k) d -> u (k d)", k=8)
            nc.gpsimd.dma_gather(out_ap=oct[:], in_ap=embv,
                                 idxs_ap=embidx[:, ioff: ioff + sblk * 8],
                                 num_idxs=sblk * 128, num_idxs_reg=sblk * 128,
                                 elem_size=256)
            f0 = gat.tile([128, sblk, 64], F32, tag="f0", bufs=2)
            nc.gpsimd.dma_gather(out_ap=f0[:], in_ap=d["feats"][:],
                                 idxs_ap=n0i[:, ioff: ioff + sblk * 8],
                                 num_idxs=sblk * 128, num_idxs_reg=sblk * 128,
                                 elem_size=64)
            f1 = gat.tile([128, sblk, 64], F32, tag="f1", bufs=2)
            nc.gpsimd.dma_gather(out_ap=f1[:], in_ap=d["feats"][:],
                                 idxs_ap=n1i[:, ioff: ioff + sblk * 8],
                                 num_idxs=sblk * 128, num_idxs_reg=sblk * 128,
                                 elem_size=64)
            df = gat.tile([128, sblk, 64], F32, tag="df", bufs=2)
            nc.vector.tensor_tensor(df[:], f0[:], f1[:], op=AL.subtract)

            for ci in range(CPS):
                gc = seg * CPS + ci
                cs = slice(gc * 512, (gc + 1) * 512)
                # ---- sub-row select: memb = (iota==sub) * octet ----
                mmk = wk.tile([128, 4, 256], F32, tag="mmk", bufs=2)
                for k in range(4):
                    blki = ci * 4 + k
                    nc.vector.scalar_tensor_tensor(
                        mmk[:, k, :], iota128[:],
                        subt[:, b * nblk + so + blki: b * nblk + so + blki + 1],
                        oct[:, blki, :], op0=AL.is_equal, op1=AL.mult)
                # ---- transposes ----
                pmT = ps.tile([128, 2, 512], F32, tag="pb", bufs=3)
                p64a = ps.tile([64, 512], F32, tag="pa", bufs=2)
                p64b = ps.tile([64, 512], F32, tag="pa", bufs=2)
                for k in range(4):
                    blki = ci * 4 + k
                    for h in range(2):
                        nc.tensor.matmul(pmT[:, h, k * 128:(k + 1) * 128],
                                         mmk[:, k, h * 128:(h + 1) * 128],
                                         ident[:], is_transpose=True,
                                         start=(k == 0), stop=(k == 3))
                    nc.tensor.matmul(p64a[:, k * 128:(k + 1) * 128], f1[:, blki, :],
                                     ident[:], is_transpose=True,
                                     start=(k == 0), stop=(k == 3))
                    nc.tensor.matmul(p64b[:, k * 128:(k + 1) * 128], df[:, blki, :],
                                     ident[:], is_transpose=True,
                                     start=(k == 0), stop=(k == 3))
                mT = tloc.tile([128, 2, 512], F32R, tag="mT")
                nc.scalar.copy(mT[:], pmT[:])
                f1t = tloc.tile([64, 512], F32R, tag="f1t")
                nc.scalar.copy(f1t[:], p64a[:])
                dft = tloc.tile([64, 512], F32R, tag="dft")
                nc.scalar.copy(dft[:], p64b[:])

                # ---- projections ----
                qep = ps.tile([128, 2, 512], F32, tag="pb", bufs=3)
                dtp = ps.tile([128, 2, 512], F32, tag="pb", bufs=3)
                for j in range(2):
                    for kb in range(2):
                        nc.tensor.matmul(qep[:, j, :], w32s(kb, b, 0, j),
                                         mT[:, kb, :], start=(kb == 0), stop=(kb == 1))
                    nc.tensor.matmul(dtp[:, j, :], w64s(b, 0, j), dft[:],
                                     start=True, stop=True)
                dts = wk.tile([128, 2, 512], F32, tag="wf", bufs=4)
                nc.scalar.copy(dts[:], dtp[:])
                pp = wk.tile([128, 2, 512], BF16, tag="wh", bufs=3)
                nc.vector.tensor_tensor(pp[:], qep[:], dts[:], op=AL.mult)
                ldc = ps.tile([4, 512], F32, tag="pa", bufs=2)
                for blk in range(2):
                    nc.tensor.matmul(ldc[:], hmb[:, blk * 4:(blk + 1) * 4],
                                     pp[:, blk, :], start=(blk == 0), stop=(blk == 1))
                a0 = wk.tile([4, 512], BF16, tag="a0", bufs=1)
                nc.scalar.activation(a0[:], ldc[:], ACTF.Sigmoid, scale=0.125)

                ktp = ps.tile([128, 2, 512], F32, tag="pb", bufs=3)
                pre0 = ps.tile([128, 2, 512], F32, tag="pb", bufs=3)
                for j in range(2):
                    for kb in range(2):
                        nc.tensor.matmul(ktp[:, j, :], w32s(kb, b, 1, j),
                                         mT[:, kb, :], start=(kb == 0), stop=(kb == 1))
                        nc.tensor.matmul(pre0[:, j, :], w32s(kb, b, 0, j),
                                         mT[:, kb, :], start=(kb == 0), stop=False)
                    nc.tensor.matmul(pre0[:, j, :], w64s(b, 0, j), f1t[:],
                                     start=False, stop=True)
                k0c = wk.tile([128, 2, 512], BF16, tag="wh", bufs=3)
                nc.scalar.copy(k0c[:], ktp[:])
                nc.sync.dma_start(k0d[:, :, cs], k0c[:])
                aep = ps.tile([128, 2, 512], F32, tag="pb", bufs=3)
                for blk in range(2):
                    nc.tensor.matmul(aep[:, blk, :], exb[:, blk * 128:(blk + 1) * 128],
                                     a0[:], start=True, stop=True)
                tt = wk.tile([128, 2, 512], F32, tag="wf", bufs=4)
                nc.vector.tensor_tensor(tt[:], aep[:], dts[:], op=AL.mult)
                pre = wk.tile([128, 2, 512], F32, tag="wf", bufs=4)
                nc.vector.tensor_tensor(pre[:], tt[:], pre0[:], op=AL.add)
                npre = wk.tile([128, 2, 512], F32, tag="wf", bufs=4)
                nc.vector.tensor_scalar_min(npre[:], pre[:], 0.0)
                epre = wk.tile([128, 2, 512], F32, tag="wf", bufs=4)
                nc.scalar.activation(epre[:], npre[:], ACTF.Exp)
                em1 = wk.tile([128, 2, 512], F32, tag="wf", bufs=4)
                nc.vector.tensor_scalar_add(em1[:], epre[:], -1.0)
                e1 = wk.tile([128, 2, 512], BF16, tag="wh", bufs=3)
                nc.vector.scalar_tensor_tensor(e1[:], pre[:], 0.0, em1[:],
                                               op0=AL.max, op1=AL.add)
                k1p = ps.tile([128, 2, 512], F32, tag="pb", bufs=3)
                for j in range(2):
                    for kb in range(2):
                        nc.tensor.matmul(k1p[:, j, :], w256s(kb, b, 0, j),
                                         e1[:, kb, :],
                                         start=(kb == 0), stop=(kb == 1))
                k1c = wk.tile([128, 2, 512], BF16, tag="wh", bufs=3)
                nc.scalar.copy(k1c[:], k1p[:])
                nc.sync.dma_start(k1d[:, :, cs], k1c[:])

        # ================= node layers =================
        GRP = 1024                       # slots per batched vector group
        NGR = slots // GRP
        for l in range(2):
            qp = ps.tile([128, 2, rpc], F32, tag="pb", bufs=3)
            if l == 0:
                for j in range(2):
                    nc.tensor.matmul(qp[:, j, :], w64s(b, 1, j), ftt[:],
                                     start=True, stop=True)
            else:
                for j in range(2):
                    for kb in range(2):
                        nc.tensor.matmul(qp[:, j, :], w256s(kb, b, 1, j),
                                         f12[b][0][:, kb, :],
                                         start=(kb == 0), stop=(kb == 1))
            qf = wk.tile([128, 2, rpc], F32, tag="qf", bufs=1)
            nc.scalar.copy(qf[:], qp[:])
            qh = wk.tile([128, 2, rpc], BF16, tag="qh", bufs=1)
            nc.vector.tensor_copy(qh[:], qf[:])

            kd = k0d if l == 0 else k1d
            def kt_group(g):
                gs = slice(g * GRP, (g + 1) * GRP)
                kt = wk.tile([128, 2, GRP], BF16, tag="ktc", bufs=3)
                nc.sync.dma_start(kt[:], kd[:, :, gs])
                return kt[:]

            ln = ps.tile([NP, 512], F32, tag="pa", bufs=2)
            for g in range(NGR):
                ktv = kt_group(g)
                rw = GRP // 16
                qb = qh[:, :, g * rw:(g + 1) * rw].unsqueeze(3).to_broadcast(
                    [128, 2, rw, 16])
                pr = wk.tile([128, 2, GRP], BF16, tag="pp2", bufs=2)
                nc.vector.tensor_tensor(
                    pr[:].rearrange("p b (r s) -> p b r s", s=16), qb,
                    ktv.rearrange("p b (r s) -> p b r s", s=16), op=AL.mult)
                for cc in range(GRP // 512):
                    c = g * (GRP // 512) + cc
                    for blk in range(2):
                        nc.tensor.matmul(ln[:], hms(blk, c),
                                         pr[:, blk, cc * 512:(cc + 1) * 512],
                                         start=(c == 0 and blk == 0),
                                         stop=(c == nch - 1 and blk == 1))

            lnv = ln[:].rearrange("p (r s) -> p r s", s=16)
            mx = wk.tile([NP, 32], F32, tag="mx", bufs=2)
            nc.vector.tensor_reduce(mx[:], lnv, axis=mybir.AxisListType.X, op=AL.max)
            sub = wk.tile([NP, 512], F32, tag="wf", bufs=4)
            nc.vector.tensor_tensor(sub[:].rearrange("p (r s) -> p r s", s=16), lnv,
                                    mx[:].unsqueeze(2).to_broadcast([NP, 32, 16]),
                                    op=AL.subtract)
            esub = wk.tile([NP, 512], F32, tag="wf", bufs=4)
            nc.scalar.activation(esub[:], sub[:], ACTF.Exp, scale=0.125)
            zs = wk.tile([NP, 32], F32, tag="zs", bufs=2)
            nc.vector.tensor_reduce(zs[:], esub[:].rearrange("p (r s) -> p r s", s=16),
                                    axis=mybir.AxisListType.X, op=AL.add)
            zr = wk.tile([NP, 32], F32, tag="zr", bufs=2)
            nc.vector.reciprocal(zr[:], zs[:])
            pn = wk.tile([NP, 512], BF16, tag="pn", bufs=2)
            nc.vector.tensor_tensor(pn[:].rearrange("p (r s) -> p r s", s=16),
                                    esub[:].rearrange("p (r s) -> p r s", s=16),
                                    zr[:].unsqueeze(2).to_broadcast([NP, 32, 16]),
                                    op=AL.mult)

            agg = wk.tile([128, 2, rpc], F32, tag="agg", bufs=1)
            for g in range(NGR):
                aeh = wk.tile([128, 2, GRP], BF16, tag="aeh2", bufs=2)
                for cc in range(GRP // 512):
                    c = g * (GRP // 512) + cc
                    aep = ps.tile([128, 2, 512], F32, tag="pb", bufs=3)
                    for blk in range(2):
                        nc.tensor.matmul(aep[:, blk, :], exs(blk, c), pn[:],
                                         start=True, stop=True)
                    nc.scalar.copy(aeh[:, :, cc * 512:(cc + 1) * 512], aep[:])
                ak = wk.tile([128, 2, GRP], BF16, tag="pp2", bufs=2)
                nc.vector.tensor_tensor(ak[:], aeh[:], kt_group(g), op=AL.mult)
                rw = GRP // 16
                nc.vector.tensor_reduce(agg[:, :, g * rw:(g + 1) * rw],
                                        ak[:].rearrange("p b (r s) -> p b r s", s=16),
                                        axis=mybir.AxisListType.X, op=AL.add)

            prn = wk.tile([128, 2, rpc], F32, tag="wf", bufs=4)
            nc.vector.tensor_tensor(prn[:], qf[:], agg[:], op=AL.add)
            npre = wk.tile([128, 2, rpc], F32, tag="wf", bufs=4)
            nc.vector.tensor_scalar_min(npre[:], prn[:], 0.0)
            epre = wk.tile([128, 2, rpc], F32, tag="wf", bufs=4)
            nc.scalar.activation(epre[:], npre[:], ACTF.Exp)
            em1 = wk.tile([128, 2, rpc], F32, tag="wf", bufs=4)
            nc.vector.tensor_scalar_add(em1[:], epre[:], -1.0)
            fl = big.tile([128, 2, rpc], BF16, tag=f"f12_{b}_{l}")
            nc.vector.scalar_tensor_tensor(fl[:], prn[:], 0.0, em1[:],
                                           op0=AL.max, op1=AL.add)
            f12[b][l] = fl

    # ================= finale =================
    gsb = []
    for b in range(NMP):
        tp = ps.tile([128, rpc], F32, tag="pb", bufs=3)
        for kb in range(4):
            l, jb = kb // 2, kb % 2
            nc.tensor.matmul(tp[:], wg[:, kb * 128:(kb + 1) * 128],
                             f12[b][l][:, jb, :], start=(kb == 0), stop=(kb == 3))
        th = wk.tile([128, rpc], F32R, tag="wf", bufs=4)
        nc.scalar.activation(th[:], tp[:], ACTF.Tanh)
        gp = ps.tile([1, rpc], F32, tag="pa", bufs=2)
        nc.tensor.matmul(gp[:], vg[:], th[:], start=True, stop=True)
        g = wk.tile([1, rpc], F32, tag=f"g{b}")
        nc.scalar.copy(g[:], gp[:])
        gsb.append(g)

    gd = wk.tile([1, rpc], F32, tag="gd")
    nc.vector.tensor_tensor(gd[:], gsb[0][:], gsb[1][:], op=AL.subtract)
    gate0 = wk.tile([1, rpc], F32, tag="gate0")
    nc.scalar.activation(gate0[:], gd[:], ACTF.Sigmoid)
    gate1 = wk.tile([1, rpc], F32, tag="gate1")
    nc.vector.tensor_scalar(gate1[:], gate0[:], -1.0, 1.0, op0=AL.mult, op1=AL.add)
    nc.sync.dma_start(gate_o[0:1, :], gate0[:])
    nc.sync.dma_start(gate_o[1:2, :], gate1[:])

    ge = []
    g0r = wk.tile([1, rpc], F32R, tag="g0r")
    nc.scalar.copy(g0r[:], gate0[:])
    g1r = wk.tile([1, rpc], F32R, tag="g1r")
    nc.scalar.copy(g1r[:], gate1[:])
    for b in range(NMP):
        gep = ps.tile([128, rpc], F32, tag="pb", bufs=3)
        nc.tensor.matmul(gep[:], ones1[:],
                         g0r[:] if b == 0 else g1r[:], start=True, stop=True)
        gs = wk.tile([128, rpc], BF16, tag=f"ge{b}")
        nc.scalar.copy(gs[:], gep[:])
        ge.append(gs)
    pooled = [None, None]
    for l in range(2):
        t0 = wk.tile([128, 2, rpc], BF16, tag="wh", bufs=3)
        nc.vector.tensor_tensor(t0[:], ge[0][:].unsqueeze(1).to_broadcast([128, 2, rpc]),
                                f12[0][l][:], op=AL.mult)
        t1 = wk.tile([128, 2, rpc], BF16, tag="wh", bufs=3)
        nc.vector.tensor_tensor(t1[:], ge[1][:].unsqueeze(1).to_broadcast([128, 2, rpc]),
                                f12[1][l][:], op=AL.mult)
        pl = wk.tile([128, 2, rpc], BF16, tag=f"pool{l}")
        nc.vector.tensor_tensor(pl[:], t0[:], t1[:], op=AL.add)
        pooled[l] = pl

    lg = ps.tile([NCLS, rpc], F32, tag="pa", bufs=2)
    for kb in range(4):
        l, jb = kb // 2, kb % 2
        nc.tensor.matmul(lg[:], wfc[:, kb * NCLS:(kb + 1) * NCLS],
                         pooled[l][:, jb, :], start=(kb == 0), stop=(kb == 3))
    lgb = wk.tile([NCLS, rpc], F32, tag="lgb")
    nc.vector.tensor_scalar_add(lgb[:], lg[:], bfc[:, 0:1])

    rbk = rpc // 128
    lgT = wk.tile([128, rbk * NCLS], F32, tag="lgT")
    plg = ps.tile([128, rbk * NCLS], F32, tag="pa", bufs=2)
    for k in range(rbk):
        nc.tensor.matmul(plg[:, k * NCLS:(k + 1) * NCLS], lgb[:, k * 128:(k + 1) * 128],
                         ident[0:NCLS, 0:NCLS], is_transpose=True,
                         start=(k == 0), stop=(k == rbk - 1))
    nc.scalar.copy(lgT[:], plg[:])
    nc.sync.dma_start(logits_o[:].rearrange("(k p) c -> p k c", p=128),
                      lgT[:].rearrange("p (k c) -> p k c", c=NCLS))

    ctx.close()


def host_prep(inputs, rpc=RPC, n_cores=N_CORES):
    feats = np.ascontiguousarray(np.asarray(inputs["feats"], np.float32))
    emb = np.ascontiguousarray(np.asarray(inputs["edge_emb"], np.float32))
    tid = np.asarray(inputs["train_ids"]).astype(np.int64)
    n2e = np.asarray(inputs["node2edge_idx"]).astype(np.int64)
    adj = np.asarray(inputs["edge_node_adj"]).astype(np.int64)

    def arr(k):
        return np.asarray(inputs[k], np.float32)

    w32 = np.zeros((128, 2 * NMP * 2 * 256), np.float32)
    w64 = np.zeros((64, NMP * 2 * 256), np.float32)
    w256 = np.zeros((128, 2 * NMP * 2 * 256), np.float32)  # cast to bf16 below
    for b in range(NMP):
        prep_w = arr("edge_prep_w")[b]
        for i, wmat in enumerate([prep_w @ _wflat(arr("edge_wq_l0")[b]),
                                  prep_w @ _wflat(arr("node_wk_l0")[b])]):
            wstk = np.tile(wmat, (8, 1))          # [256, 256]
            for kb in range(2):
                base = ((kb * NMP + b) * 2 + i) * 256
                w32[:, base:base + 256] = wstk[kb * 128:(kb + 1) * 128]
        w64[:, (b * 2 + 0) * 256:(b * 2 + 1) * 256] = arr("W_prep1") @ _wflat(arr("edge_wk_l0")[b])
        w64[:, (b * 2 + 1) * 256:(b * 2 + 2) * 256] = arr("W_prep0") @ _wflat(arr("node_wq_l0")[b])
        wnk1 = _wflat(arr("node_wk_l1")[b])
        wq1 = _wflat(arr("node_wq_l1")[b])
        for kb in range(2):
            w256[:, ((kb * NMP + b) * 2 + 0) * 256:((kb * NMP + b) * 2 + 1) * 256] = \
                wnk1[kb * 128:(kb + 1) * 128]
            w256[:, ((kb * NMP + b) * 2 + 1) * 256:((kb * NMP + b) * 2 + 2) * 256] = \
                wq1[kb * 128:(kb + 1) * 128]
    import ml_dtypes
    wg = np.ascontiguousarray(arr("Wg").reshape(4, 128, 128).transpose(1, 0, 2)
                              .reshape(128, 4 * 128)).astype(ml_dtypes.bfloat16)
    vg = arr("vg").reshape(128, 1)
    wfc = np.ascontiguousarray(arr("W_fc").reshape(4, 128, NCLS).transpose(1, 0, 2)
                               .reshape(128, 4 * NCLS)).astype(ml_dtypes.bfloat16)
    bfc = arr("b_fc").reshape(NCLS, 1)
    hmb, exb, hm, exc = build_host_consts(rpc)

    w256 = w256.astype(ml_dtypes.bfloat16)
    shared = dict(feats=feats, edge_emb=emb, w32=w32, w64=w64, w256=w256,
                  wg=wg, vg=vg, wfc=wfc, bfc=bfc, hmb=hmb, exb=exb, hm=hm, ex=exc,
                  ones1=np.ones((1, 128), np.float32),
                  iota128=np.tile(np.arange(256) // 32, (128, 1)).astype(np.float32))

    slots = rpc * S
    nblk = slots // 128
    sblk = nblk // NSEG
    in_maps = []
    for c in range(n_cores):
        rows = tid[c * rpc:(c + 1) * rpc]
        embidx = np.zeros((128, NMP * (slots // 16)), np.int16)
        subv = np.zeros((128, NMP * nblk), np.float32)
        n0w = np.zeros((128, NMP * (slots // 16)), np.int16)
        n1w = np.zeros((128, NMP * (slots // 16)), np.int16)
        for b in range(NMP):
            eidx = n2e[b][rows].reshape(-1)
            a = adj[b][eidx]
            subv[:, b * nblk:(b + 1) * nblk] = (eidx & 7).reshape(nblk, 128).T
            for seg in range(NSEG):
                lo = seg * sblk * 128
                hi = (seg + 1) * sblk * 128
                sl = slice(b * (slots // 16) + seg * sblk * 8,
                           b * (slots // 16) + (seg + 1) * sblk * 8)
                embidx[:, sl] = _wrap16_rep(eidx[lo:hi] >> 3)
                n0w[:, sl] = _wrap16_rep(a[lo:hi, 0])
                n1w[:, sl] = _wrap16_rep(a[lo:hi, 1])
        tidw = _wrap16_rep(rows)
        m = dict(shared)
        m.update(embidx=embidx, sub=subv, n0idx=n0w, n1idx=n1w, tididx=tidw)
        in_maps.append(m)
    return in_maps


_CACHED_NC = None


def kernel(**inputs):
    global _CACHED_NC
    if _CACHED_NC is None:
        _CACHED_NC = build_program(RPC)
    nc = _CACHED_NC
    in_maps = host_prep(inputs, RPC, N_CORES)
    from concourse.bass_utils import run_bass_kernel_spmd
    res = run_bass_kernel_spmd(nc, in_maps, core_ids=list(range(N_CORES)))
    logits = np.concatenate([res.results[c]["logits_o"] for c in range(N_CORES)], axis=0)
    gate = np.concatenate([res.results[c]["gate_o"] for c in range(N_CORES)], axis=1)
    return logits.astype(np.float32), gate.astype(np.float32)
